# revision 31
# baseline (speedup 1.0000x reference)
"""LSTM layer with per-step weights on 8 trn2 NeuronCores.

Strategy: tensor-parallel over the c/h hidden dim (1024 -> 8 shards of 128).
Each core k owns rows [k*128,(k+1)*128) of the four gate weight matrices for
every step, computes its shard of the gates/cell/hidden state with the FULL
batch (128) as the matmul streaming dim, and the h shard is AllGathered
across the 8 cores between steps.  Wout is split along its output dim
(256 -> 8 slices of 32); each core computes its y slice from the gathered h.

Matmul orientation: out[c_shard=128, batch=128] = W_shard^T.T @ xh^T, i.e.
lhsT = W^T chunks [K=128, M=128] (stationary), rhs = xh^T chunks
[K=128, N=128] (streaming).  h is produced as [c_units, batch] which is
exactly the rhs layout needed by the next step -- no transposes anywhere.

Weights/activations in bf16 (PE runs fp32 matmul at 1/4 rate), cell state
and all elementwise math in fp32.
"""

import numpy as np
import ml_dtypes

T, B, IN, H, O = 24, 128, 512, 1024, 256
NCORES = 8
SH = H // NCORES  # 128 c/h units per core
OS = O // NCORES  # 32 output channels per core
KC = (IN + H) // 128  # 12 contraction chunks (4 from x, 8 from h)
N_DUMMY = 22  # keep-warm filler matmuls per AllGather window (HAM clock gate)

BF16 = ml_dtypes.bfloat16

_cached = {}


def _build_module():
    import concourse.bacc as bacc
    import concourse.tile as tile
    import concourse.mybir as mybir

    f32 = mybir.dt.float32
    bf16 = mybir.dt.bfloat16
    AF = mybir.ActivationFunctionType
    ALU = mybir.AluOpType

    nc = bacc.Bacc(
        "TRN2",
        target_bir_lowering=False,
        debug=False,
        enable_asserts=False,
        num_devices=NCORES,
    )

    wt_d = nc.dram_tensor("wt", [T, 128, 4 * KC * 128], bf16, kind="ExternalInput")
    xt_d = nc.dram_tensor("xt", [T, 128, IN], bf16, kind="ExternalInput")
    wo_d = nc.dram_tensor("wo", [T, 128, 8 * OS], bf16, kind="ExternalInput")
    bias_d = nc.dram_tensor("bias", [128, 4 * T], f32, kind="ExternalInput")
    bout_d = nc.dram_tensor("bout", [OS, T], f32, kind="ExternalInput")
    h0_d = nc.dram_tensor("h0", [128, H], bf16, kind="ExternalInput")
    c0_d = nc.dram_tensor("c0", [128, B], f32, kind="ExternalInput")
    y_d = nc.dram_tensor("y", [OS, T * B], f32, kind="ExternalOutput")

    with tile.TileContext(nc) as tc:
        with (
            tc.tile_pool(name="const", bufs=1) as cpool,
            tc.tile_pool(name="w", bufs=4) as wpool,
            tc.tile_pool(name="h", bufs=3) as hpool,
            tc.tile_pool(name="gates", bufs=2) as gpool,
            tc.tile_pool(name="tmp", bufs=2) as tpool,
            tc.tile_pool(name="yout", bufs=3) as ypool,
            tc.tile_pool(name="ps", bufs=8, space="PSUM") as pspool,
            tc.tile_pool(name="dram", bufs=3, space="DRAM") as dpool,
        ):
            # resident tensors (bulk preloads on the scalar ring; sync ring is
            # reserved for the per-step weight stream)
            x_sb = cpool.tile([128, T * IN], bf16)
            xv = x_sb[:].rearrange("p (t n) -> p t n", t=T)
            xsrc = xt_d[:].rearrange("t p n -> p t n")
            nc.scalar.dma_start(xv[:, 0:4, :], xsrc[:, 0:4, :])
            nc.scalar.dma_start(xv[:, 4:T, :], xsrc[:, 4:T, :])
            wo_sb = cpool.tile([128, T * 8 * OS], bf16)
            nc.scalar.dma_start(
                wo_sb[:].rearrange("p (t n) -> p t n", t=T),
                wo_d[:].rearrange("t p n -> p t n"),
            )
            bias_sb = cpool.tile([128, 4 * T], f32)
            nc.scalar.dma_start(bias_sb[:], bias_d[:])
            bout_sb = cpool.tile([OS, T], f32)
            nc.scalar.dma_start(bout_sb[:], bout_d[:])
            y_all = cpool.tile([OS, T * B], f32)

            c_sb = cpool.tile([128, B], f32)
            nc.scalar.dma_start(c_sb[:], c0_d[:])
            h_prev = hpool.tile([128, H], bf16)
            nc.scalar.dma_start(h_prev[:], h0_d[:])

            def load_w(t):
                # SWDGE on gpsimd: emitted after the collective trigger so the
                # bulk HBM read never contends with the latency-critical hsh
                # write that gates the trigger
                w = wpool.tile([128, 4 * KC * 128], bf16, name=f"w{t}", tag="w")
                nc.scalar.dma_start(w[:], wt_d[t])
                return w

            def xpart(t, w, after=None):
                from concourse.tile_rust import add_dep_helper

                ps = [
                    pspool.tile([128, B], f32, tag="ps", name=f"ps{t}_{g}")
                    for g in range(4)
                ]
                first = None
                last = None
                for g in range(4):
                    for kc in range(4):
                        col = (g * KC + kc) * 128
                        mm = nc.tensor.matmul(
                            ps[g][:],
                            w[:, col : col + 128],
                            x_sb[:, t * IN + kc * 128 : t * IN + (kc + 1) * 128],
                            start=(kc == 0),
                            stop=False,
                        )
                        if first is None:
                            first = mm
                        last = mm
                if after is not None:
                    add_dep_helper(first.ins, after.ins, sync=False,
                                   reason="xpart after keep-warm filler")
                return ps, last

            # prologue: step 0 weights + x-part
            w_cur = load_w(0)
            ps, xp_last = xpart(0, w_cur)

            for t in range(T):
                # h-part of gates(t) -- waits on the gathered h(t-1)
                for g in range(4):
                    for kc in range(4, KC):
                        col = (g * KC + kc) * 128
                        nc.tensor.matmul(
                            ps[g][:],
                            w_cur[:, col : col + 128],
                            h_prev[:, (kc - 4) * 128 : (kc - 3) * 128],
                            start=False,
                            stop=(kc == KC - 1),
                        )

                # gate activations (bias folded in)
                zt = gpool.tile([128, B], f32, tag="zt")
                nc.scalar.activation(zt[:], ps[0][:], AF.Tanh, bias=bias_sb[:, 4 * t : 4 * t + 1])
                it = gpool.tile([128, B], f32, tag="it")
                nc.scalar.activation(it[:], ps[1][:], AF.Sigmoid, bias=bias_sb[:, 4 * t + 1 : 4 * t + 2])
                ft = gpool.tile([128, B], f32, tag="ft")
                nc.scalar.activation(ft[:], ps[2][:], AF.Sigmoid, bias=bias_sb[:, 4 * t + 2 : 4 * t + 3])
                ot = gpool.tile([128, B], f32, tag="ot")
                nc.scalar.activation(ot[:], ps[3][:], AF.Sigmoid, bias=bias_sb[:, 4 * t + 3 : 4 * t + 4])

                # c = f*c + i*z ; hn = o * tanh(c)
                t1 = tpool.tile([128, B], f32, tag="t1")
                nc.vector.scalar_tensor_tensor(t1[:], zt[:], 0.0, it[:], ALU.bypass, ALU.mult)
                t2 = tpool.tile([128, B], f32, tag="t2")
                nc.vector.scalar_tensor_tensor(t2[:], c_sb[:], 0.0, ft[:], ALU.bypass, ALU.mult)
                nc.vector.scalar_tensor_tensor(c_sb[:], t1[:], 0.0, t2[:], ALU.bypass, ALU.add)
                tcn = tpool.tile([128, B], f32, tag="tcn")
                nc.scalar.activation(tcn[:], c_sb[:], AF.Tanh)
                hn = tpool.tile([128, B], bf16, tag="hn")
                nc.vector.scalar_tensor_tensor(hn[:], tcn[:], 0.0, ot[:], ALU.bypass, ALU.mult)

                # AllGather h shard -> full h for next step + Wout
                hsh = dpool.tile([128, B], bf16, tag="hsh")
                nc.sync.dma_start(hsh[:], hn[:])
                hg = dpool.tile([NCORES * 128, B], bf16, tag="hg", addr_space="Shared")
                nc.gpsimd.collective_compute(
                    "AllGather",
                    ALU.bypass,
                    replica_groups=[list(range(NCORES))],
                    ins=[hsh.opt()],
                    outs=[hg.opt()],
                )

                # prefetch next step's weights + x-part matmuls + keep-warm
                # filler: all of it runs on PE inside the AllGather window.
                # The filler's lhsT tile is memset by gpsimd right after the
                # trigger, so the filler cannot start (and burn its clock
                # budget) before the collective is actually in flight.
                if t + 1 < T:
                    gate = cpool.tile([128, 128], bf16, tag="gate", name=f"gate{t}", bufs=2)
                    nc.gpsimd.memset(gate[:], 0.0)
                    w_next = load_w(t + 1)
                    dscr = pspool.tile([128, 512], f32, tag="ps", name=f"dscr{t}")
                    dlast = None
                    for dd in range(N_DUMMY):
                        dlast = nc.tensor.matmul(
                            dscr[:],
                            gate[:],
                            x_sb[:, 0:512],
                            start=True,
                            stop=True,
                        )
                    ps_next, xp_last = xpart(t + 1, w_next, after=dlast)
                else:
                    w_next, ps_next, xp_last = None, None, None

                h_new = hpool.tile([128, H], bf16, name=f"hnew{t}", tag="h")
                # merged strided DMAs split across both HWDGE rings; scalar's
                # ring is free right after the AG, sync's drains the w stream
                hv = h_new[:].rearrange("p (c n) -> p c n", c=NCORES)
                gv = hg[:].rearrange("(c p) n -> p c n", p=128)
                nc.scalar.dma_start(hv[:, 0:4, :], gv[:, 0:4, :])
                nc.sync.dma_start(hv[:, 4:8, :], gv[:, 4:8, :])

                # y_t slice = sigmoid(Wout_slice @ h_t + bout_slice)
                from concourse.tile_rust import add_dep_helper

                yps = pspool.tile([OS, B], f32, tag="ps")
                for kc in range(8):
                    col = t * 8 * OS + kc * OS
                    mm = nc.tensor.matmul(
                        yps[:],
                        wo_sb[:, col : col + OS],
                        h_new[:, kc * 128 : (kc + 1) * 128],
                        start=(kc == 0),
                        stop=(kc == 7),
                    )
                    if kc == 0 and xp_last is not None:
                        add_dep_helper(mm.ins, xp_last.ins, sync=False,
                                       reason="Wout after window filler")
                nc.scalar.activation(
                    y_all[:, t * B : (t + 1) * B], yps[:], AF.Sigmoid,
                    bias=bout_sb[:, t : t + 1],
                )

                h_prev = h_new
                w_cur = w_next
                ps = ps_next

            nc.sync.dma_start(y_d[:], y_all[:])

    nc.compile()
    return nc


def _prep_inputs(x, W, Wi, Wf, Wo, Wout, b, bi, bf, bo, bout, c0, h0):
    """Build the 8 per-core input maps (host-side layout shuffling)."""
    # xt[t, p, kc*128 + bb] = x[bb, t, kc*128+p]   (shared by all cores)
    xt = (
        x.transpose(1, 2, 0)
        .reshape(T, 4, 128, B)
        .transpose(0, 2, 1, 3)
        .reshape(T, 128, IN)
    )
    xt = np.ascontiguousarray(xt).astype(BF16)

    Wall = np.stack([W, Wi, Wf, Wo], axis=1)  # [T, 4, H, IN+H]
    Ball = np.stack([b, bi, bf, bo], axis=1)  # [T, 4, H]

    h0t = h0.reshape(NCORES, 128).T  # [128, 8]
    h0b = np.ascontiguousarray(
        np.broadcast_to(h0t[:, :, None], (128, NCORES, B)).reshape(128, H)
    ).astype(BF16)

    in_maps = []
    for k in range(NCORES):
        sh = slice(k * SH, (k + 1) * SH)
        osl = slice(k * OS, (k + 1) * OS)
        # wt[t, p, (g*KC+kc)*128+m] = Wall[t, g, sh.start+m, kc*128+p]
        wt = (
            Wall[:, :, sh, :]
            .reshape(T, 4, SH, KC, 128)
            .transpose(0, 4, 1, 3, 2)
            .reshape(T, 128, 4 * KC * 128)
        )
        wt = np.ascontiguousarray(wt).astype(BF16)
        # wo[t, p, kc*OS+m] = Wout[t, osl.start+m, kc*128+p]
        wo = (
            Wout[:, osl, :]
            .transpose(0, 2, 1)
            .reshape(T, 8, 128, OS)
            .transpose(0, 2, 1, 3)
            .reshape(T, 128, 8 * OS)
        )
        wo = np.ascontiguousarray(wo).astype(BF16)
        bias = np.ascontiguousarray(
            Ball[:, :, sh].transpose(2, 0, 1).reshape(128, 4 * T)
        ).astype(np.float32)
        bout_k = np.ascontiguousarray(bout[:, osl].T).astype(np.float32)
        c0b = np.ascontiguousarray(
            np.broadcast_to(c0[sh][:, None], (128, B))
        ).astype(np.float32)
        in_maps.append(
            {
                "wt": wt,
                "xt": xt,
                "wo": wo,
                "bias": bias,
                "bout": bout_k,
                "h0": h0b,
                "c0": c0b,
            }
        )
    return in_maps


def kernel(**inputs):
    from concourse.bass_utils import run_bass_kernel_spmd

    inputs = {k: np.asarray(v, dtype=np.float32) for k, v in inputs.items()}
    in_maps = _prep_inputs(**inputs)

    if "nc" not in _cached:
        _cached["nc"] = _build_module()
    nc = _cached["nc"]

    res = run_bass_kernel_spmd(nc, in_maps, core_ids=list(range(NCORES)))
    ys = [r["y"].reshape(OS, T, B) for r in res.results]  # each [OS, T, B]
    Y = np.stack(ys, axis=0)  # [8, OS, T, B]
    out = Y.transpose(3, 2, 0, 1).reshape(B, T * O)
    return np.ascontiguousarray(out).astype(np.float32)


if __name__ == "__main__":
    rng = np.random.default_rng(0)
    ih = IN + H
    ins = {
        "x": rng.standard_normal((B, T, IN), dtype=np.float32),
        "W": rng.standard_normal((T, H, ih), dtype=np.float32) * 0.02,
        "Wi": rng.standard_normal((T, H, ih), dtype=np.float32) * 0.02,
        "Wf": rng.standard_normal((T, H, ih), dtype=np.float32) * 0.02,
        "Wo": rng.standard_normal((T, H, ih), dtype=np.float32) * 0.02,
        "Wout": rng.standard_normal((T, O, H), dtype=np.float32) * 0.02,
        "b": rng.standard_normal((T, H), dtype=np.float32) * 0.02,
        "bi": rng.standard_normal((T, H), dtype=np.float32) * 0.02,
        "bf": rng.standard_normal((T, H), dtype=np.float32) * 0.02,
        "bo": rng.standard_normal((T, H), dtype=np.float32) * 0.02,
        "bout": rng.standard_normal((T, O), dtype=np.float32) * 0.02,
        "c0": rng.standard_normal((H,), dtype=np.float32) * 0.02,
        "h0": rng.standard_normal((H,), dtype=np.float32) * 0.02,
    }
    out = kernel(**ins)
    print("kernel output", out.shape, out.dtype, float(np.abs(out).mean()))


# revision 33
# speedup vs baseline: 1.1395x; 1.1395x over previous
"""LSTM layer with per-step weights on 8 trn2 NeuronCores.

Strategy: tensor-parallel over the c/h hidden dim (1024 -> 8 shards of 128).
Each core k owns rows [k*128,(k+1)*128) of the four gate weight matrices for
every step, computes its shard of the gates/cell/hidden state with the FULL
batch (128) as the matmul streaming dim, and the h shard is AllGathered
across the 8 cores between steps.  Wout is split along its output dim
(256 -> 8 slices of 32); each core computes its y slice from the gathered h.

Matmul orientation: out[c_shard=128, batch=128] = W_shard^T.T @ xh^T, i.e.
lhsT = W^T chunks [K=128, M=128] (stationary), rhs = xh^T chunks
[K=128, N=128] (streaming).  h is produced as [c_units, batch] which is
exactly the rhs layout needed by the next step -- no transposes anywhere.

Weights/activations in bf16 (PE runs fp32 matmul at 1/4 rate), cell state
and all elementwise math in fp32.
"""

import numpy as np
import ml_dtypes

T, B, IN, H, O = 24, 128, 512, 1024, 256
NCORES = 8
SH = H // NCORES  # 128 c/h units per core
OS = O // NCORES  # 32 output channels per core
KC = (IN + H) // 128  # 12 contraction chunks (4 from x, 8 from h)
N_DUMMY = 22  # keep-warm filler matmuls per AllGather window (HAM clock gate)

BF16 = ml_dtypes.bfloat16

_cached = {}


def _build_module():
    import concourse.bacc as bacc
    import concourse.tile as tile
    import concourse.mybir as mybir

    f32 = mybir.dt.float32
    bf16 = mybir.dt.bfloat16
    AF = mybir.ActivationFunctionType
    ALU = mybir.AluOpType

    nc = bacc.Bacc(
        "TRN2",
        target_bir_lowering=False,
        debug=False,
        enable_asserts=False,
        num_devices=NCORES,
    )

    wt_d = nc.dram_tensor("wt", [T, 128, 4 * KC * 128], bf16, kind="ExternalInput")
    xt_d = nc.dram_tensor("xt", [T, 128, IN], bf16, kind="ExternalInput")
    wo_d = nc.dram_tensor("wo", [T, 128, 8 * OS], bf16, kind="ExternalInput")
    bias_d = nc.dram_tensor("bias", [128, 4 * T], f32, kind="ExternalInput")
    bout_d = nc.dram_tensor("bout", [OS, T], f32, kind="ExternalInput")
    h0_d = nc.dram_tensor("h0", [128, H], bf16, kind="ExternalInput")
    c0_d = nc.dram_tensor("c0", [128, B], f32, kind="ExternalInput")
    y_d = nc.dram_tensor("y", [OS, T * B], f32, kind="ExternalOutput")

    with tile.TileContext(nc) as tc:
        with (
            tc.tile_pool(name="const", bufs=1) as cpool,
            tc.tile_pool(name="w", bufs=9) as wpool,
            tc.tile_pool(name="h", bufs=3) as hpool,
            tc.tile_pool(name="gates", bufs=2) as gpool,
            tc.tile_pool(name="tmp", bufs=2) as tpool,
            tc.tile_pool(name="yout", bufs=3) as ypool,
            tc.tile_pool(name="ps", bufs=8, space="PSUM") as pspool,
            tc.tile_pool(name="dram", bufs=3, space="DRAM") as dpool,
        ):
            # resident tensors (bulk preloads on the scalar ring; sync ring is
            # reserved for the per-step weight stream)
            x_sb = cpool.tile([128, T * IN], bf16)
            xv = x_sb[:].rearrange("p (t n) -> p t n", t=T)
            xsrc = xt_d[:].rearrange("t p n -> p t n")
            nc.scalar.dma_start(xv[:, 0:4, :], xsrc[:, 0:4, :])
            nc.scalar.dma_start(xv[:, 4:T, :], xsrc[:, 4:T, :])
            wo_sb = cpool.tile([128, T * 8 * OS], bf16)
            nc.scalar.dma_start(
                wo_sb[:].rearrange("p (t n) -> p t n", t=T),
                wo_d[:].rearrange("t p n -> p t n"),
            )
            bias_sb = cpool.tile([128, 4 * T], f32)
            nc.scalar.dma_start(bias_sb[:], bias_d[:])
            bout_sb = cpool.tile([OS, T], f32)
            nc.scalar.dma_start(bout_sb[:], bout_d[:])
            y_all = cpool.tile([OS, T * B], f32)

            c_sb = cpool.tile([128, B], f32)
            nc.scalar.dma_start(c_sb[:], c0_d[:])
            h_prev = hpool.tile([128, H], bf16)
            nc.scalar.dma_start(h_prev[:], h0_d[:])

            def load_w(t):
                # SWDGE on gpsimd: emitted after the collective trigger so the
                # bulk HBM read never contends with the latency-critical hsh
                # write that gates the trigger
                w = wpool.tile([128, 4 * KC * 128], bf16, name=f"w{t}", tag="w")
                nc.sync.dma_start(w[:], wt_d[t])
                return w

            def xpart(t, w, after=None):
                from concourse.tile_rust import add_dep_helper

                ps = [
                    pspool.tile([128, B], f32, tag="ps", name=f"ps{t}_{g}")
                    for g in range(4)
                ]
                first = None
                last = None
                for g in range(4):
                    for kc in range(4):
                        col = (g * KC + kc) * 128
                        mm = nc.tensor.matmul(
                            ps[g][:],
                            w[:, col : col + 128],
                            x_sb[:, t * IN + kc * 128 : t * IN + (kc + 1) * 128],
                            start=(kc == 0),
                            stop=False,
                        )
                        if first is None:
                            first = mm
                        last = mm
                if after is not None:
                    add_dep_helper(first.ins, after.ins, sync=False,
                                   reason="xpart after keep-warm filler")
                return ps, last

            # prologue: step 0 weights + x-part
            w_cur = load_w(0)
            ps, xp_last = xpart(0, w_cur)

            for t in range(T):
                # h-part of gates(t) -- waits on the gathered h(t-1)
                for g in range(4):
                    for kc in range(4, KC):
                        col = (g * KC + kc) * 128
                        nc.tensor.matmul(
                            ps[g][:],
                            w_cur[:, col : col + 128],
                            h_prev[:, (kc - 4) * 128 : (kc - 3) * 128],
                            start=False,
                            stop=(kc == KC - 1),
                        )

                # gate activations (bias folded in)
                zt = gpool.tile([128, B], f32, tag="zt")
                nc.scalar.activation(zt[:], ps[0][:], AF.Tanh, bias=bias_sb[:, 4 * t : 4 * t + 1])
                it = gpool.tile([128, B], f32, tag="it")
                nc.scalar.activation(it[:], ps[1][:], AF.Sigmoid, bias=bias_sb[:, 4 * t + 1 : 4 * t + 2])
                ft = gpool.tile([128, B], f32, tag="ft")
                nc.scalar.activation(ft[:], ps[2][:], AF.Sigmoid, bias=bias_sb[:, 4 * t + 2 : 4 * t + 3])
                ot = gpool.tile([128, B], f32, tag="ot")
                nc.scalar.activation(ot[:], ps[3][:], AF.Sigmoid, bias=bias_sb[:, 4 * t + 3 : 4 * t + 4])

                # c = f*c + i*z ; hn = o * tanh(c)
                t1 = tpool.tile([128, B], f32, tag="t1")
                nc.vector.scalar_tensor_tensor(t1[:], zt[:], 0.0, it[:], ALU.bypass, ALU.mult)
                t2 = tpool.tile([128, B], f32, tag="t2")
                nc.vector.scalar_tensor_tensor(t2[:], c_sb[:], 0.0, ft[:], ALU.bypass, ALU.mult)
                nc.vector.scalar_tensor_tensor(c_sb[:], t1[:], 0.0, t2[:], ALU.bypass, ALU.add)
                tcn = tpool.tile([128, B], f32, tag="tcn")
                nc.scalar.activation(tcn[:], c_sb[:], AF.Tanh)
                hn = tpool.tile([128, B], bf16, tag="hn")
                nc.vector.scalar_tensor_tensor(hn[:], tcn[:], 0.0, ot[:], ALU.bypass, ALU.mult)

                # AllGather h shard -> full h for next step + Wout
                hsh = dpool.tile([128, B], bf16, tag="hsh")
                nc.sync.dma_start(hsh[:], hn[:])
                hg = dpool.tile([NCORES * 128, B], bf16, tag="hg", addr_space="Shared")
                nc.gpsimd.collective_compute(
                    "AllGather",
                    ALU.bypass,
                    replica_groups=[list(range(NCORES))],
                    ins=[hsh.opt()],
                    outs=[hg.opt()],
                )

                # prefetch next step's weights + x-part matmuls + keep-warm
                # filler: all of it runs on PE inside the AllGather window.
                # The filler's lhsT tile is memset by gpsimd right after the
                # trigger, so the filler cannot start (and burn its clock
                # budget) before the collective is actually in flight.
                if t + 1 < T:
                    gate = cpool.tile([128, 128], bf16, tag="gate", name=f"gate{t}", bufs=2)
                    nc.gpsimd.memset(gate[:], 0.0)
                    w_next = load_w(t + 1)
                    dscr = pspool.tile([128, 512], f32, tag="ps", name=f"dscr{t}")
                    dlast = None
                    for dd in range(N_DUMMY):
                        dlast = nc.tensor.matmul(
                            dscr[:],
                            gate[:],
                            x_sb[:, 0:512],
                            start=True,
                            stop=True,
                        )
                    ps_next, xp_last = xpart(t + 1, w_next, after=dlast)
                else:
                    w_next, ps_next, xp_last = None, None, None

                h_new = hpool.tile([128, H], bf16, name=f"hnew{t}", tag="h")
                # merged strided DMAs split across both HWDGE rings; scalar's
                # ring is free right after the AG, sync's drains the w stream
                hv = h_new[:].rearrange("p (c n) -> p c n", c=NCORES)
                gv = hg[:].rearrange("(c p) n -> p c n", p=128)
                nc.scalar.dma_start(hv[:, 0:4, :], gv[:, 0:4, :])
                nc.sync.dma_start(hv[:, 4:8, :], gv[:, 4:8, :])

                # y_t slice = sigmoid(Wout_slice @ h_t + bout_slice)
                from concourse.tile_rust import add_dep_helper

                yps = pspool.tile([OS, B], f32, tag="ps")
                for kc in range(8):
                    col = t * 8 * OS + kc * OS
                    mm = nc.tensor.matmul(
                        yps[:],
                        wo_sb[:, col : col + OS],
                        h_new[:, kc * 128 : (kc + 1) * 128],
                        start=(kc == 0),
                        stop=(kc == 7),
                    )
                    if kc == 0 and xp_last is not None:
                        add_dep_helper(mm.ins, xp_last.ins, sync=False,
                                       reason="Wout after window filler")
                nc.scalar.activation(
                    y_all[:, t * B : (t + 1) * B], yps[:], AF.Sigmoid,
                    bias=bout_sb[:, t : t + 1],
                )

                h_prev = h_new
                w_cur = w_next
                ps = ps_next

            nc.sync.dma_start(y_d[:], y_all[:])

    nc.compile()
    return nc


def _prep_inputs(x, W, Wi, Wf, Wo, Wout, b, bi, bf, bo, bout, c0, h0):
    """Build the 8 per-core input maps (host-side layout shuffling)."""
    # xt[t, p, kc*128 + bb] = x[bb, t, kc*128+p]   (shared by all cores)
    xt = (
        x.transpose(1, 2, 0)
        .reshape(T, 4, 128, B)
        .transpose(0, 2, 1, 3)
        .reshape(T, 128, IN)
    )
    xt = np.ascontiguousarray(xt).astype(BF16)

    Wall = np.stack([W, Wi, Wf, Wo], axis=1)  # [T, 4, H, IN+H]
    Ball = np.stack([b, bi, bf, bo], axis=1)  # [T, 4, H]

    h0t = h0.reshape(NCORES, 128).T  # [128, 8]
    h0b = np.ascontiguousarray(
        np.broadcast_to(h0t[:, :, None], (128, NCORES, B)).reshape(128, H)
    ).astype(BF16)

    in_maps = []
    for k in range(NCORES):
        sh = slice(k * SH, (k + 1) * SH)
        osl = slice(k * OS, (k + 1) * OS)
        # wt[t, p, (g*KC+kc)*128+m] = Wall[t, g, sh.start+m, kc*128+p]
        wt = (
            Wall[:, :, sh, :]
            .reshape(T, 4, SH, KC, 128)
            .transpose(0, 4, 1, 3, 2)
            .reshape(T, 128, 4 * KC * 128)
        )
        wt = np.ascontiguousarray(wt).astype(BF16)
        # wo[t, p, kc*OS+m] = Wout[t, osl.start+m, kc*128+p]
        wo = (
            Wout[:, osl, :]
            .transpose(0, 2, 1)
            .reshape(T, 8, 128, OS)
            .transpose(0, 2, 1, 3)
            .reshape(T, 128, 8 * OS)
        )
        wo = np.ascontiguousarray(wo).astype(BF16)
        bias = np.ascontiguousarray(
            Ball[:, :, sh].transpose(2, 0, 1).reshape(128, 4 * T)
        ).astype(np.float32)
        bout_k = np.ascontiguousarray(bout[:, osl].T).astype(np.float32)
        c0b = np.ascontiguousarray(
            np.broadcast_to(c0[sh][:, None], (128, B))
        ).astype(np.float32)
        in_maps.append(
            {
                "wt": wt,
                "xt": xt,
                "wo": wo,
                "bias": bias,
                "bout": bout_k,
                "h0": h0b,
                "c0": c0b,
            }
        )
    return in_maps


def kernel(**inputs):
    from concourse.bass_utils import run_bass_kernel_spmd

    inputs = {k: np.asarray(v, dtype=np.float32) for k, v in inputs.items()}
    in_maps = _prep_inputs(**inputs)

    if "nc" not in _cached:
        _cached["nc"] = _build_module()
    nc = _cached["nc"]

    res = run_bass_kernel_spmd(nc, in_maps, core_ids=list(range(NCORES)))
    ys = [r["y"].reshape(OS, T, B) for r in res.results]  # each [OS, T, B]
    Y = np.stack(ys, axis=0)  # [8, OS, T, B]
    out = Y.transpose(3, 2, 0, 1).reshape(B, T * O)
    return np.ascontiguousarray(out).astype(np.float32)


if __name__ == "__main__":
    rng = np.random.default_rng(0)
    ih = IN + H
    ins = {
        "x": rng.standard_normal((B, T, IN), dtype=np.float32),
        "W": rng.standard_normal((T, H, ih), dtype=np.float32) * 0.02,
        "Wi": rng.standard_normal((T, H, ih), dtype=np.float32) * 0.02,
        "Wf": rng.standard_normal((T, H, ih), dtype=np.float32) * 0.02,
        "Wo": rng.standard_normal((T, H, ih), dtype=np.float32) * 0.02,
        "Wout": rng.standard_normal((T, O, H), dtype=np.float32) * 0.02,
        "b": rng.standard_normal((T, H), dtype=np.float32) * 0.02,
        "bi": rng.standard_normal((T, H), dtype=np.float32) * 0.02,
        "bf": rng.standard_normal((T, H), dtype=np.float32) * 0.02,
        "bo": rng.standard_normal((T, H), dtype=np.float32) * 0.02,
        "bout": rng.standard_normal((T, O), dtype=np.float32) * 0.02,
        "c0": rng.standard_normal((H,), dtype=np.float32) * 0.02,
        "h0": rng.standard_normal((H,), dtype=np.float32) * 0.02,
    }
    out = kernel(**ins)
    print("kernel output", out.shape, out.dtype, float(np.abs(out).mean()))
